# revision 1
# baseline (speedup 1.0000x reference)
"""Single-head causal attention (B=4, T=2048, C=1024, H=64) on 8 NeuronCores.

Sharding: 8 cores = 4 batches x 2 interleaved halves. Core (b, h) computes
query blocks of 512 rows: h=0 -> rows [0:512] and [1024:1536]; h=1 -> rows
[512:1024] and [1536:2048]. This balances causal work while keeping ONE SPMD
program: all per-core differences enter through input DATA.

Causality, with zero per-chunk instructions:
  - the score matmuls contract over K=66: rows 0:64 are the head dim, rows
    64:65 of the key operand hold per-(block, chunk) biases (0 or -1e30/scale)
    and the query operand holds block-selector rows (1/0). Acausal chunks thus
    come out of the matmul pre-biased to -1e30 and exp() kills them for free.
  - diagonal (partially causal) chunks are masked post-exp with slices of one
    shared staircase tile on DVE.

Layout: scores are computed transposed (scoresT[tk, tq]) so softmax sums come
from the PV matmul itself: V is augmented with a ones column -> PV psum row 64
is the denominator. Matmul path is bf16 (inputs rounded on host); psum
accumulation and the normalize/store epilogue are fp32.

All DRAM inputs are host-prepacked to the exact SBUF layout so every big DMA
is one contiguous descriptor per partition; DMA issue is spread over the
sync + scalar HWDGE sequencers (constants via gpsimd SWDGE). V-transposes are
interleaved into the attention loops to keep the PE HAM-warm.
"""

import numpy as np
import ml_dtypes

import concourse.bass as bass
from concourse import bacc
import concourse.mybir as mybir
import concourse.tile as tile
from concourse.bass_utils import run_bass_kernel_spmd

B, T, C, H = 4, 2048, 1024, 64
P = 128
TQ = 512                 # query block width
NBLK = 2                 # query blocks per core
NQ = NBLK * TQ           # 1024 query rows per core
SCHED = (4, 12)          # full-phase k-chunks per block (compile-time max)
NDIAG = TQ // P          # 4 diagonal chunks per block
KFULL = SCHED[-1] * P    # 1536 k columns needed for full phase
NKCH = KFULL // TQ       # 3 xk column chunks
CCH = C // P             # 8 contraction chunks
NV = NDIAG * NBLK + SCHED[-1]   # 8 diag + 12 full v blocks of 128 rows
SCALE = float(C) ** -0.5
BIGNEG = -1e30 / SCALE   # lands as -1e30 after the exp scale

F32 = mybir.dt.float32
BF16 = mybir.dt.bfloat16
NPBF = ml_dtypes.bfloat16

_CACHE = {}


def build():
    nc = bacc.Bacc()
    xq_d = nc.declare_dram_parameter("xq", [NBLK, P, CCH * TQ], BF16, isOutput=False)
    xk_d = nc.declare_dram_parameter("xk", [NKCH, P, CCH * TQ], BF16, isOutput=False)
    wkv_d = nc.declare_dram_parameter("wkv", [P, CCH * 2 * H], BF16, isOutput=False)
    wqv_d = nc.declare_dram_parameter("wqv", [P, CCH * 2 * H], BF16, isOutput=False)
    wk_d = nc.declare_dram_parameter("wk", [P, CCH * H], BF16, isOutput=False)
    aug_d = nc.declare_dram_parameter("aug", [2, NQ + KFULL + NQ], BF16, isOutput=False)
    st_d = nc.declare_dram_parameter("stair", [P, 896], BF16, isOutput=False)
    idb_d = nc.declare_dram_parameter("identb", [P, P], BF16, isOutput=False)
    idf_d = nc.declare_dram_parameter("identf", [P, P], F32, isOutput=False)
    on_d = nc.declare_dram_parameter("vones", [P, NV], BF16, isOutput=False)
    out_d = nc.declare_dram_parameter("out", [P, NBLK * NDIAG * H], F32, isOutput=True)

    EXPF = mybir.ActivationFunctionType.Exp

    with tile.TileContext(nc) as tc:
        with (
            tc.tile_pool(name="big", bufs=1) as big,
            tc.tile_pool(name="work", bufs=4) as work,
            tc.tile_pool(name="ps", bufs=2, space="PSUM") as psp,
            tc.tile_pool(name="ps_s", bufs=2, space="PSUM") as pss,
            tc.tile_pool(name="ps_pv", bufs=1, space="PSUM") as pspv,
            tc.tile_pool(name="ps_tr", bufs=1, space="PSUM") as pstr,
        ):
            # ---- DMAs: weights first (they gate the first matmuls), x chunks
            # next, spread over both HWDGE sequencers; constants via SWDGE ----
            wqv = big.tile([P, CCH, 2 * H], BF16)
            nc.sync.dma_start(out=wqv[:], in_=wqv_d[:].rearrange("p (nc h) -> p nc h", nc=CCH))
            wkv = big.tile([P, CCH, 2 * H], BF16)
            nc.scalar.dma_start(out=wkv[:], in_=wkv_d[:].rearrange("p (nc h) -> p nc h", nc=CCH))
            wk = big.tile([P, CCH, H], BF16)
            nc.scalar.dma_start(out=wk[:], in_=wk_d[:].rearrange("p (nc h) -> p nc h", nc=CCH))
            xqs, xks = [], []
            for i in range(NBLK):
                t = big.tile([P, CCH, TQ], BF16, tag=f"xq{i}")
                [nc.sync, nc.scalar][i % 2].dma_start(
                    out=t[:], in_=xq_d[i].rearrange("p (nc t) -> p nc t", nc=CCH))
                xqs.append(t)
            for i in range(NKCH):
                t = big.tile([P, CCH, TQ], BF16, tag=f"xk{i}")
                [nc.scalar, nc.sync, nc.sync][i].dma_start(
                    out=t[:], in_=xk_d[i].rearrange("p (nc t) -> p nc t", nc=CCH))
                xks.append(t)
            # augmented-contraction rows: qb selectors / ktb biases / kdb zeros
            qb = big.tile([66, NQ], BF16)
            nc.gpsimd.dma_start(out=qb[64:66, :], in_=aug_d[:, 0:NQ])
            ktb = big.tile([66, KFULL], BF16)
            nc.gpsimd.dma_start(out=ktb[64:66, :], in_=aug_d[:, NQ:NQ + KFULL])
            kdb = big.tile([66, NQ], BF16)
            nc.gpsimd.dma_start(out=kdb[64:66, :], in_=aug_d[:, NQ + KFULL:])
            stair = big.tile([P, 896], BF16)
            nc.gpsimd.dma_start(out=stair[:], in_=st_d[:])
            identb = big.tile([P, P], BF16)
            nc.gpsimd.dma_start(out=identb[:], in_=idb_d[:])
            identf = big.tile([P, P], F32)
            nc.gpsimd.dma_start(out=identf[:], in_=idf_d[:])
            vtmp = big.tile([P, NV], BF16)
            nc.gpsimd.dma_start(out=vtmp[:], in_=on_d[:])

            # ---- projections ----
            # qvd: rows 0:64 = qT, rows 64:128 = vdiagT (from xq); q is then
            # copied into qb whose rows 64:66 hold the block selectors.
            vdh = big.tile([P, NQ], BF16)
            for i in range(NBLK):
                ps = psp.tile([P, TQ], F32, tag="proj")
                for cc in range(CCH):
                    nc.tensor.matmul(ps[:], wqv[:, cc, :], xqs[i][:, cc, :],
                                     start=(cc == 0), stop=(cc == CCH - 1))
                nc.vector.tensor_copy(vdh[:, bass.ts(i, TQ)], ps[:])
                nc.vector.tensor_copy(qb[0:64, bass.ts(i, TQ)], vdh[0:64, bass.ts(i, TQ)])
            # kdT: diag-key projection (from xq) into kdb rows 0:64
            for i in range(NBLK):
                ps = psp.tile([64, TQ], F32, tag="proj")
                for cc in range(CCH):
                    nc.tensor.matmul(ps[:], wk[:, cc, :], xqs[i][:, cc, :],
                                     start=(cc == 0), stop=(cc == CCH - 1))
                nc.vector.tensor_copy(kdb[0:64, bass.ts(i, TQ)], ps[:])
            # kv: rows 0:64 = kT, rows 64:128 = vT (full prefix, from xk);
            # k is then copied into ktb whose rows 64:66 hold the biases.
            kvh = big.tile([P, KFULL], BF16)
            for i in range(NKCH):
                ps = psp.tile([P, TQ], F32, tag="proj")
                for cc in range(CCH):
                    nc.tensor.matmul(ps[:], wkv[:, cc, :], xks[i][:, cc, :],
                                     start=(cc == 0), stop=(cc == CCH - 1))
                nc.vector.tensor_copy(kvh[:, bass.ts(i, TQ)], ps[:])
                nc.vector.tensor_copy(ktb[0:64, bass.ts(i, TQ)], kvh[0:64, bass.ts(i, TQ)])

            # ---- v_aug tiles: [128, 65] per 128-row block, col 64 = 1.0 ----
            vaug = big.tile([P, NV, H + 1], BF16)
            nc.vector.tensor_copy(vaug[:, :, H], vtmp[:])

            def make_vaug(slot, src_upper, col0):
                # transpose vT[64, col0:col0+128] (in partitions 64:128 of
                # src_upper) -> vaug[:, slot, 0:64]
                tp = pstr.tile([P, H], BF16, tag="tr")
                nc.tensor.transpose(tp[:], src_upper[64:128, col0:col0 + P],
                                    identb[64:128, 64:128])
                nc.vector.tensor_copy(vaug[:, slot, 0:H], tp[:])

            # ---- attention (v-transposes interleaved to keep PE warm) ----
            vmade = set()
            for blk in range(NBLK):
                pv = pspv.tile([96, TQ], F32, tag="pv")
                qT = qb[0:66, bass.ts(blk, TQ)]
                chunks = ([("d", d) for d in range(NDIAG)] +
                          [("f", c) for c in range(SCHED[blk])])
                nmm = len(chunks)
                for g0 in range(0, nmm, 2):
                    grp = chunks[g0:g0 + 2]
                    s = pss.tile([P, len(grp) * TQ], F32, tag="s")
                    for gi, (kind, c) in enumerate(grp):
                        slot = blk * NDIAG + c if kind == "d" else NBLK * NDIAG + c
                        if slot not in vmade:
                            vmade.add(slot)
                            if kind == "d":
                                make_vaug(slot, vdh, blk * TQ + c * P)
                            else:
                                make_vaug(slot, kvh, c * P)
                        lhsT = (kdb[:, blk * TQ + c * P: blk * TQ + (c + 1) * P]
                                if kind == "d" else ktb[:, bass.ts(c, P)])
                        nc.tensor.matmul(s[:, bass.ts(gi, TQ)], lhsT, qT,
                                         start=True, stop=True)
                    e = work.tile([P, len(grp) * TQ], BF16, tag="e")
                    nc.scalar.activation(e[:], s[:], EXPF, scale=SCALE)
                    for gi, (kind, c) in enumerate(grp):
                        if kind == "d":
                            off = 384 - 128 * c
                            nc.vector.tensor_mul(e[:, bass.ts(gi, TQ)],
                                                 e[:, bass.ts(gi, TQ)],
                                                 stair[:, off:off + TQ])
                    for gi, (kind, c) in enumerate(grp):
                        slot = blk * NDIAG + c if kind == "d" else NBLK * NDIAG + c
                        mi = g0 + gi
                        nc.tensor.matmul(pv[0:H + 1, :], vaug[:, slot, :],
                                         e[:, bass.ts(gi, TQ)],
                                         start=(mi == 0), stop=(mi == nmm - 1))

                # ---- epilogue (fp32): transpose, divide, store ----
                pvs = work.tile([96, TQ], F32, tag="pvs")
                nc.vector.tensor_copy(pvs[0:H + 1, :], pv[0:H + 1, :])
                ob = work.tile([P, NDIAG, H], F32, tag="ob")
                for j in range(NDIAG):
                    ot = pstr.tile([P, 96], F32, tag="tr")
                    nc.tensor.transpose(ot[:], pvs[:, bass.ts(j, P)], identf[0:96, 0:96])
                    r = work.tile([P, 1], F32, tag="r")
                    nc.vector.reciprocal(r[:], ot[:, H:H + 1])
                    nc.vector.tensor_scalar_mul(ob[:, j, :], ot[:, 0:H], r[:])
                nc.sync.dma_start(out=out_d[:, blk * NDIAG * H:(blk + 1) * NDIAG * H],
                                  in_=ob[:])
    nc.compile()
    return nc


def _pack_x(xT, cols):
    # xT: [C, T] fp32 -> [P, CCH*W] bf16 in SBUF layout
    a = xT[:, cols]                                   # [C, W]
    a = a.reshape(CCH, P, -1).transpose(1, 0, 2)      # [P, CCH, W]
    return np.ascontiguousarray(a.reshape(P, -1)).astype(NPBF)


def _pack_w(w):
    # w: [C, width] -> [P, CCH*width]
    a = w.reshape(CCH, P, -1).transpose(1, 0, 2)
    return np.ascontiguousarray(a.reshape(P, -1)).astype(NPBF)


def _host_inputs(x, Wk, Wq, Wv):
    wkv = _pack_w(np.concatenate([Wk, Wv], axis=1))
    wqv = _pack_w(np.concatenate([Wq, Wv], axis=1))
    wk = _pack_w(Wk)
    ii = np.arange(P)
    stair = (np.arange(896)[None, :] >= ii[:, None] + 384).astype(NPBF)
    identb = np.eye(P, dtype=NPBF)
    identf = np.eye(P, dtype=np.float32)
    vones = np.ones((P, NV), NPBF)
    # block-selector rows for qb: row r is 1 on block r's columns
    qaug = np.zeros((2, NQ), np.float32)
    qaug[0, :TQ] = 1.0
    qaug[1, TQ:] = 1.0
    in_maps = []
    for b in range(B):
        xT = np.ascontiguousarray(x[b].T.astype(np.float32))  # [C, T]
        for h in range(2):
            q0s = (0, 1024) if h == 0 else (512, 1536)
            xq = np.stack([_pack_x(xT, slice(q0, q0 + TQ)) for q0 in q0s])
            xk = np.stack([_pack_x(xT, slice(i * TQ, (i + 1) * TQ))
                           for i in range(NKCH)])
            # ktb bias rows: row blk, col t = 0 if chunk t//128 is a (strictly
            # pre-diagonal) causal chunk for this core's block blk, else BIGNEG
            kaug = np.full((2, KFULL), BIGNEG, np.float32)
            for blk, q0 in enumerate(q0s):
                kaug[blk, :q0] = 0.0
            aug = np.concatenate(
                [qaug, kaug, np.zeros((2, NQ), np.float32)], axis=1).astype(NPBF)
            in_maps.append(dict(xq=xq, xk=xk, wkv=wkv, wqv=wqv, wk=wk,
                                aug=aug, stair=stair, identb=identb,
                                identf=identf, vones=vones))
    return in_maps


def kernel(x, Wk, Wq, Wv, trace=False):
    x = np.asarray(x, np.float32)
    in_maps = _host_inputs(x, np.asarray(Wk, np.float32),
                           np.asarray(Wq, np.float32), np.asarray(Wv, np.float32))
    if "nc" not in _CACHE:
        _CACHE["nc"] = build()
    nc = _CACHE["nc"]
    res = run_bass_kernel_spmd(nc, in_maps, list(range(8)), trace=trace)
    out = np.empty((B, T, H), np.float32)
    for b in range(B):
        for h in range(2):
            o = res.results[b * 2 + h]["out"]  # [P, NBLK*NDIAG*H]
            o = np.asarray(o).reshape(P, NBLK, NDIAG, H)
            q0s = (0, 1024) if h == 0 else (512, 1536)
            for blk, q0 in enumerate(q0s):
                # row q0 + j*128 + p  <-  o[p, blk, j, :]
                out[b, q0:q0 + TQ] = o[:, blk].transpose(1, 0, 2).reshape(TQ, H)
    kernel.last_exec_time_ns = res.exec_time_ns
    kernel.last_results = res
    return out



# revision 6
# speedup vs baseline: 1.3562x; 1.3562x over previous
"""Single-head causal attention (B=4, T=2048, C=1024, H=64) on 8 NeuronCores.

Sharding: 8 cores = 4 batches x 2 interleaved halves. Core (b, h) computes
query blocks of 512 rows: blk0 = rows [h*512, h*512+512), blk1 = rows
[1024+h*512, 1024+h*512+512).

v2 design vs the previous kernel:
  - ONE x layout of 4 slots of 512 rows per core: slot0 = blk0 query rows,
    slot1 = blk1 query rows, slot2/slot3 = the remaining prefix rows (per-core
    DATA chooses which; h=0's slot3 is dead padding killed by the bias rows).
    The k/v projection of the slots serves BOTH the full-phase keys and the
    diagonal keys, eliminating the separate diag-key projection and the
    duplicated x DMA of the old xq/xk split (4MB vs 5MB).
  - PE p-state management: TRN2's PE runs at 1.2GHz until ~3us of continuous
    execution, 2.4GHz after. A warmup chain of matmuls on a zero tile runs
    during the DMA lead-in so real matmuls start at full clock, and the PE
    program order is arranged to have no long waits afterwards.
  - DMA: slots are split in half across the sync+scalar HWDGE queues in
    arrival-priority order (wkv/wq first, then xs0, xs2, xs1, xs3); constants
    ride the gpsimd SWDGE queue.
  - Causality is data-driven as before: kt rows 64:66 hold per-(block, chunk)
    biases (0 or -1e30/scale), qb rows 64:66 hold block selectors, diagonal
    chunks are masked post-exp with slices of a shared staircase tile.
  - Scores are computed transposed (scoresT[tk, tq]); V is augmented with a
    ones column so PV psum row 64 is the softmax denominator.
  - exp() runs on the scalar engine (the only engine with activation); the PE
    emission order interleaves projections/transposes into the activation
    latency so neither engine starves.
"""

import numpy as np
import ml_dtypes

import concourse.bass as bass
from concourse import bacc
import concourse.mybir as mybir
import concourse.tile as tile
from concourse.bass_utils import run_bass_kernel_spmd

B, T, C, H = 4, 2048, 1024, 64
P = 128
TQ = 512                 # rows per slot / query block width
NSLOT = 4
CCH = C // P             # 8 contraction chunks
NDIAG = TQ // P          # 4 chunks of 128 per slot
SCALE = float(C) ** -0.5
BIGNEG = -1e30 / SCALE   # lands as -1e30 after the exp scale
NWARM = 22               # PE warmup matmuls (p-state ramp during DMA lead-in)

F32 = mybir.dt.float32
BF16 = mybir.dt.bfloat16
NPBF = ml_dtypes.bfloat16

_CACHE = {}

# chunk schedule: per block, list of (slot, c, kind). Uniform across cores.
CHUNKS = {
    0: [(0, c, "d") for c in range(NDIAG)] + [(2, c, "f") for c in range(NDIAG)],
    1: [(0, c, "f") for c in range(NDIAG)] + [(2, c, "f") for c in range(NDIAG)]
       + [(1, c, "d") for c in range(NDIAG)] + [(3, c, "f") for c in range(NDIAG)],
}
NGRP = {0: len(CHUNKS[0]) // 2, 1: len(CHUNKS[1]) // 2}


def build():
    nc = bacc.Bacc()
    xs_d = nc.declare_dram_parameter("xs", [NSLOT, P, CCH * TQ], BF16, isOutput=False)
    wkv_d = nc.declare_dram_parameter("wkv", [P, CCH * 2 * H], BF16, isOutput=False)
    wq_d = nc.declare_dram_parameter("wq", [P, CCH * H], BF16, isOutput=False)
    kaug_d = nc.declare_dram_parameter("kaug", [2, NSLOT * TQ], BF16, isOutput=False)
    qsel_d = nc.declare_dram_parameter("qsel", [2, 2 * TQ], BF16, isOutput=False)
    st_d = nc.declare_dram_parameter("stair", [P, 896], BF16, isOutput=False)
    idb_d = nc.declare_dram_parameter("identb", [P, P], BF16, isOutput=False)
    idf_d = nc.declare_dram_parameter("identf", [H + 1, H + 1], F32, isOutput=False)
    on_d = nc.declare_dram_parameter("vones", [P, NSLOT * NDIAG], BF16, isOutput=False)
    out_d = nc.declare_dram_parameter("out", [P, 2 * NDIAG * H], F32, isOutput=True)

    EXPF = mybir.ActivationFunctionType.Exp
    KW = NSLOT * TQ  # 2048 key columns

    with tile.TileContext(nc) as tc:
        with (
            tc.tile_pool(name="big", bufs=1) as big,
            tc.tile_pool(name="work", bufs=4) as work,
            tc.tile_pool(name="pw", bufs=2, space="PSUM") as pw,
            tc.tile_pool(name="pss", bufs=2, space="PSUM") as pss,
            tc.tile_pool(name="ps_pv", bufs=1, space="PSUM") as pspv,
            tc.tile_pool(name="ps_tr", bufs=1, space="PSUM") as pstr,
        ):
            # ---- DMA issues, priority order per queue ----
            wkv = big.tile([P, CCH, 2 * H], BF16)
            nc.sync.dma_start(out=wkv[:], in_=wkv_d[:].rearrange("p (nc h) -> p nc h", nc=CCH))
            wq = big.tile([P, CCH, H], BF16)
            nc.scalar.dma_start(out=wq[:], in_=wq_d[:].rearrange("p (nc h) -> p nc h", nc=CCH))
            xts = []
            for s in range(NSLOT):
                xts.append(big.tile([P, CCH, TQ], BF16, tag=f"xs{s}", name=f"xs{s}"))
            HC = CCH // 2
            for s in (0, 2, 1, 3):   # arrival priority
                t = xts[s]
                nc.sync.dma_start(
                    out=t[:, 0:HC, :],
                    in_=xs_d[s, :, 0:HC * TQ].rearrange("p (nc t) -> p nc t", nc=HC))
                nc.scalar.dma_start(
                    out=t[:, HC:CCH, :],
                    in_=xs_d[s, :, HC * TQ:].rearrange("p (nc t) -> p nc t", nc=HC))
            # constants on the gpsimd SWDGE queue
            kt = big.tile([66, KW], BF16)
            nc.gpsimd.dma_start(out=kt[64:66, :], in_=kaug_d[:])
            qb = big.tile([66, 2 * TQ], BF16)
            nc.gpsimd.dma_start(out=qb[64:66, :], in_=qsel_d[:])
            identb = big.tile([P, P], BF16)
            nc.gpsimd.dma_start(out=identb[:], in_=idb_d[:])
            vtmp = big.tile([P, NSLOT * NDIAG], BF16)
            nc.gpsimd.dma_start(out=vtmp[:], in_=on_d[:])
            stair = big.tile([P, 896], BF16)
            nc.gpsimd.dma_start(out=stair[:], in_=st_d[:])
            identf = big.tile([H + 1, H + 1], F32)
            nc.gpsimd.dma_start(out=identf[:], in_=idf_d[:])

            # ---- persistent sbuf tiles ----
            vh = big.tile([P, KW], BF16)            # rows 64:128 = vT
            vaug = big.tile([P, NSLOT * NDIAG, H + 1], BF16)
            nc.vector.tensor_copy(vaug[:, :, H], vtmp[:])

            # ---- PE warmup (p-state ramp while DMA streams) ----
            zw = big.tile([P, TQ], BF16)
            nc.gpsimd.memset(zw[:], 0)
            wps = pw.tile([P, TQ], F32, tag="proj")
            for _ in range(NWARM):
                nc.tensor.matmul(wps[:], zw[:, 0:P], zw[:], start=True, stop=True)

            # ---- emission helpers ----
            def emit_kv(s):
                ps = pw.tile([P, TQ], F32, tag="proj")
                for cc in range(CCH):
                    nc.tensor.matmul(ps[:], wkv[:, cc, :], xts[s][:, cc, :],
                                     start=(cc == 0), stop=(cc == CCH - 1))
                nc.vector.tensor_copy(kt[0:64, bass.ts(s, TQ)], ps[0:64, :])
                nc.vector.tensor_copy(vh[64:128, bass.ts(s, TQ)], ps[64:128, :])

            def emit_q(blk):
                ps = pw.tile([64, TQ], F32, tag="proj")
                for cc in range(CCH):
                    nc.tensor.matmul(ps[:], wq[:, cc, :], xts[blk][:, cc, :],
                                     start=(cc == 0), stop=(cc == CCH - 1))
                nc.vector.tensor_copy(qb[0:64, bass.ts(blk, TQ)], ps[:])

            def emit_T(s):
                tp = pstr.tile([P, NDIAG, H], BF16, tag="tr")
                for c in range(NDIAG):
                    nc.tensor.transpose(tp[:, c, :],
                                        vh[64:128, s * TQ + c * P: s * TQ + (c + 1) * P],
                                        identb[64:128, 64:128])
                nc.vector.tensor_copy(vaug[:, s * NDIAG:(s + 1) * NDIAG, 0:H], tp[:])

            e_tiles = {}

            def emit_sc(blk, g):
                s = pss.tile([P, 2 * TQ], F32, tag="s")
                for gi in range(2):
                    slot, c, _ = CHUNKS[blk][2 * g + gi]
                    nc.tensor.matmul(s[:, bass.ts(gi, TQ)],
                                     kt[:, slot * TQ + c * P: slot * TQ + (c + 1) * P],
                                     qb[:, bass.ts(blk, TQ)], start=True, stop=True)
                e = work.tile([P, 2 * TQ], BF16, tag="e")
                nc.scalar.activation(e[:], s[:], EXPF, scale=SCALE)
                for gi in range(2):
                    slot, c, kind = CHUNKS[blk][2 * g + gi]
                    if kind == "d":
                        off = 384 - P * c
                        nc.vector.tensor_mul(e[:, bass.ts(gi, TQ)],
                                             e[:, bass.ts(gi, TQ)],
                                             stair[:, off:off + TQ])
                e_tiles[(blk, g)] = e

            pv_tiles = {}

            def emit_pv(blk, g):
                if g == 0:
                    pv_tiles[blk] = pspv.tile([H + 1, TQ], F32, tag="pv",
                                              name=f"pv{blk}")
                pv = pv_tiles[blk]
                e = e_tiles.pop((blk, g))
                n = len(CHUNKS[blk])
                for gi in range(2):
                    idx = 2 * g + gi
                    slot, c, _ = CHUNKS[blk][idx]
                    nc.tensor.matmul(pv[:], vaug[:, slot * NDIAG + c, :],
                                     e[:, bass.ts(gi, TQ)],
                                     start=(idx == 0), stop=(idx == n - 1))

            pvs_tiles = {}

            def emit_epi_copy(blk):
                # psum -> sbuf copy; must be emitted before the next block's
                # pv tile is allocated (pspv bufs=1)
                pvs = work.tile([H + 1, TQ], F32, tag="pvs")
                nc.vector.tensor_copy(pvs[:], pv_tiles[blk][:])
                pvs_tiles[blk] = pvs

            def emit_epi_rest(blk):
                pvs = pvs_tiles.pop(blk)
                ob = work.tile([P, NDIAG, H], F32, tag="ob")
                ot = pstr.tile([P, NDIAG, H + 1], F32, tag="tr")
                for j in range(NDIAG):
                    nc.tensor.transpose(ot[:, j, :], pvs[:, bass.ts(j, P)], identf[:])
                for j in range(NDIAG):
                    r = work.tile([P, 1], F32, tag="r")
                    nc.vector.reciprocal(r[:], ot[:, j, H:H + 1])
                    nc.vector.tensor_scalar_mul(ob[:, j, :], ot[:, j, 0:H], r[:])
                nc.sync.dma_start(out=out_d[:, blk * NDIAG * H:(blk + 1) * NDIAG * H],
                                  in_=ob[:])

            # ---- schedule (PE program order == emission order) ----
            emit_kv(0)
            emit_q(0)
            emit_T(0)
            emit_sc(0, 0)
            emit_sc(0, 1)
            emit_kv(2)
            emit_T(2)
            emit_pv(0, 0)
            emit_pv(0, 1)
            emit_sc(0, 2)
            emit_sc(0, 3)
            emit_kv(1)
            emit_q(1)
            emit_T(1)
            emit_pv(0, 2)
            emit_pv(0, 3)
            emit_epi_copy(0)
            # block 1: groups 0..7 = s0 full x2, s2 full x2, s1 diag x2, s3 full x2
            emit_sc(1, 0)
            emit_sc(1, 1)
            emit_kv(3)
            emit_sc(1, 2)
            emit_pv(1, 0)
            emit_T(3)
            emit_sc(1, 3)
            emit_pv(1, 1)
            emit_epi_rest(0)
            emit_sc(1, 4)
            emit_pv(1, 2)
            emit_sc(1, 5)
            emit_pv(1, 3)
            emit_sc(1, 6)
            emit_pv(1, 4)
            emit_sc(1, 7)
            emit_pv(1, 5)
            emit_pv(1, 6)
            emit_pv(1, 7)
            emit_epi_copy(1)
            emit_epi_rest(1)
    nc.compile()
    return nc


def _pack_x(xT, cols):
    # xT: [C, T] fp32 -> [P, CCH*W] bf16 in SBUF layout
    a = xT[:, cols]                                   # [C, W]
    a = a.reshape(CCH, P, -1).transpose(1, 0, 2)      # [P, CCH, W]
    return np.ascontiguousarray(a.reshape(P, -1)).astype(NPBF)


def _pack_w(w):
    # w: [C, width] -> [P, CCH*width]
    a = w.reshape(CCH, P, -1).transpose(1, 0, 2)
    return np.ascontiguousarray(a.reshape(P, -1)).astype(NPBF)


def _host_inputs(x, Wk, Wq, Wv):
    wkv = _pack_w(np.concatenate([Wk, Wv], axis=1))
    wq = _pack_w(Wq)
    ii = np.arange(P)
    stair = (np.arange(896)[None, :] >= ii[:, None] + 384).astype(NPBF)
    identb = np.eye(P, dtype=NPBF)
    identf = np.eye(H + 1, dtype=np.float32)
    vones = np.ones((P, NSLOT * NDIAG), NPBF)
    qsel = np.zeros((2, 2 * TQ), np.float32)
    qsel[0, :TQ] = 1.0
    qsel[1, TQ:] = 1.0
    qsel = qsel.astype(NPBF)
    in_maps = []
    for b in range(B):
        xT = np.ascontiguousarray(x[b].T.astype(np.float32))  # [C, T]
        for h in range(2):
            if h == 0:
                rows = [(0, 512), (1024, 1536), (512, 1024), (512, 1024)]
            else:
                rows = [(512, 1024), (1536, 2048), (0, 512), (1024, 1536)]
            xs = np.stack([_pack_x(xT, slice(a, bb)) for (a, bb) in rows])
            q0s = (h * TQ, 1024 + h * TQ)
            # bias rows: kaug[blk, slot cols] = 0 if slot rows fully causal
            # for that block (or the block's own diag slot), else BIGNEG
            kaug = np.full((2, NSLOT * TQ), BIGNEG, np.float32)
            for blk in range(2):
                for s, (a, bb) in enumerate(rows):
                    if s == blk:
                        kaug[blk, s * TQ:(s + 1) * TQ] = 0.0   # diag slot
                    elif bb <= q0s[blk] and not (h == 0 and s == 3):
                        kaug[blk, s * TQ:(s + 1) * TQ] = 0.0   # fully causal
            in_maps.append(dict(xs=xs, wkv=wkv, wq=wq, kaug=kaug.astype(NPBF),
                                qsel=qsel, stair=stair, identb=identb,
                                identf=identf, vones=vones))
    return in_maps


def kernel(x, Wk, Wq, Wv, trace=False):
    x = np.asarray(x, np.float32)
    in_maps = _host_inputs(x, np.asarray(Wk, np.float32),
                           np.asarray(Wq, np.float32), np.asarray(Wv, np.float32))
    if "nc" not in _CACHE:
        _CACHE["nc"] = build()
    nc = _CACHE["nc"]
    res = run_bass_kernel_spmd(nc, in_maps, list(range(8)), trace=trace)
    out = np.empty((B, T, H), np.float32)
    for b in range(B):
        for h in range(2):
            o = res.results[b * 2 + h]["out"]  # [P, 2*NDIAG*H]
            o = np.asarray(o).reshape(P, 2, NDIAG, H)
            q0s = (h * TQ, 1024 + h * TQ)
            for blk, q0 in enumerate(q0s):
                # row q0 + j*128 + p  <-  o[p, blk, j, :]
                out[b, q0:q0 + TQ] = o[:, blk].transpose(1, 0, 2).reshape(TQ, H)
    kernel.last_exec_time_ns = res.exec_time_ns
    kernel.last_results = res
    return out
